# revision 39
# baseline (speedup 1.0000x reference)
"""Trainium2 Bass kernel for the S-KalmanNet SLAM step (nn_DNN_SKalmanNet_SLAM).

Model (per branch): in -> Linear(500)+ReLU -> GRU(87) x2 -> Linear(40)+ReLU
-> Linear(25 or 4).  Two independent branches (Pk from branch 1, Sk from
branch 2).

Sharding: no useful intra-module sharding (everything is tiny), so the two
independent branches are placed on two NeuronCores (core 0 -> Pk branch,
core 1 -> Sk branch) with branch-2 shapes zero-padded to branch-1 shapes so
a single SPMD program serves both.  Cores 2-7 run the same program on
replicated data (their outputs are ignored).

All device matvecs keep vectors partition-major ([dim, 1] on SBUF
partitions).  Weights are host-side transposed and bias-augmented
([W^T; b^T]) so each matvec is a PE matmul with the weight as the
stationary operand and PSUM accumulating (W@v + b) directly.  The GRU
nonlinearity is fused onto the ACT engine (sigmoid/tanh share one
activation table set, pre-warmed by a dummy op at kernel start;
n = tanh(r*P_hn + P_in) is a single ACTIVATE via AP scale/bias, and
h' = n + z*(h-n) is three Identity ACTIVATEs kept on-engine).  Input DMAs
are spread over the SP (HWDGE) and GPSIMD (SWDGE) queues, ordered so the
L1-weight chain and the big GRU input-weight DMA arrive together.
"""

import numpy as np
from contextlib import ExitStack

X_DIM, Y_DIM, H1, H2, GH = 5, 2, 500, 40, 87
N_CORES = 8
F32 = np.float32

_RT = {}


# ---------------------------------------------------------------------------
# Device program
# ---------------------------------------------------------------------------

DMA_PLAN = [("smalls", "S"), ("wihrz", "S"), ("tail", "S"),
            ("w1t", "P"), ("whh0", "P"), ("wihn", "P")]


def build_nc(trunc=None, use_act=True, use_stt=True, use_swdge=True,
             use_warm=True, dma_plan=None):
    """Build and compile the Bass program.  trunc/use_* are debug-only
    knobs for HW bisection; dma_plan overrides input DMA queue/order."""
    import concourse.tile as tile
    from concourse import bacc, mybir
    from concourse._compat import get_trn_type

    dt = mybir.dt.float32
    AF = mybir.ActivationFunctionType
    OP = mybir.AluOpType

    nc = bacc.Bacc(get_trn_type() or "TRN2", target_bir_lowering=False,
                   debug=False, num_devices=N_CORES)

    # DRAM I/O.  wih0 is packed gate-major and split rz/n so the two DMAs
    # can ride different queues.
    smalls_d = nc.dram_tensor("smalls", [128, 3], dt, kind="ExternalInput")
    w1t_d = nc.dram_tensor("w1t", [23, 500], dt, kind="ExternalInput")
    whh0_d = nc.dram_tensor("whh0", [88, 261], dt, kind="ExternalInput")
    wihrz_d = nc.dram_tensor("wihrz", [126, 696], dt, kind="ExternalInput")
    wihn_d = nc.dram_tensor("wihn", [126, 348], dt, kind="ExternalInput")
    tail_d = nc.dram_tensor("tail", [88, 587], dt, kind="ExternalInput")
    out_d = nc.dram_tensor("out", [1, 25], dt, kind="ExternalOutput")

    with tile.TileContext(nc) as tc, ExitStack() as ctx:
        const = ctx.enter_context(tc.tile_pool(name="const", bufs=1))
        work = ctx.enter_context(tc.tile_pool(name="work", bufs=1))
        psum = ctx.enter_context(tc.tile_pool(name="psum", bufs=2,
                                              space="PSUM"))

        # --- input DMAs spread over the SP and Pool queues (ACT stays
        # free for the activation-table load + sigmoids/tanh) ------------
        need_gru = trunc != "l1"
        need_tail = trunc not in ("l1", "cell0")

        smalls = const.tile([128, 3], dt)
        w1t = const.tile([23, 500], dt)
        wihrz = const.tile([126, 696], dt)
        tail = const.tile([88, 587], dt)
        whh0 = const.tile([88, 261], dt)
        wihn = const.tile([126, 348], dt)

        tiles = {"smalls": (smalls, smalls_d), "w1t": (w1t, w1t_d),
                 "wihrz": (wihrz, wihrz_d), "tail": (tail, tail_d),
                 "whh0": (whh0, whh0_d), "wihn": (wihn, wihn_d)}
        skip = set()
        if not need_gru:
            skip |= {"wihrz", "whh0", "wihn"}
        if not need_tail:
            skip |= {"tail"}
        plan = dma_plan or DMA_PLAN
        for name, q in plan:
            if name in skip:
                continue
            eng = nc.sync if (q == "S" or not use_swdge) else nc.gpsimd
            t, d = tiles[name]
            eng.dma_start(out=t[:], in_=d.ap())

        # --- ACT table warm (sigmoid_and_others holds sigmoid AND tanh) --
        if use_warm and use_act:
            warm_in = work.tile([1, 1], dt)
            warm_out = work.tile([1, 1], dt)
            nc.vector.memset(warm_in[:], 0.0)
            nc.scalar.activation(out=warm_out[:], in_=warm_in[:],
                                 func=AF.Sigmoid)

        # --- constant-one rows (bias augmentation) ----------------------
        # DVE writes must start on a 32-aligned partition, so memset a
        # 32-aligned span covering the ones-row; the later compute op
        # overwrites the lower rows (WAW ordering keeps the 1.0 intact).
        x_sb = work.tile([126, 4], dt)      # L1 output, 4 K-chunks of 125
        nc.vector.memset(x_sb[96:126, 3:4], 1.0)
        h10 = work.tile([88, 1], dt)
        nc.vector.memset(h10[64:88, 0:1], 1.0)
        h11 = work.tile([88, 1], dt)
        nc.vector.memset(h11[64:88, 0:1], 1.0)
        p_sb = work.tile([41, 1], dt)
        nc.vector.memset(p_sb[32:41, 0:1], 1.0)

        # --- L1: x = relu(W1 @ in + b1), 4 M-chunks of 125 --------------
        ps_x = psum.tile([125, 4], dt)
        for c in range(4):
            nc.tensor.matmul(out=ps_x[0:125, c:c + 1],
                             lhsT=w1t[0:23, 125 * c:125 * c + 125],
                             rhs=smalls[0:23, 0:1], start=True, stop=True)
        nc.vector.tensor_scalar(out=x_sb[0:125, 0:4], in0=ps_x[0:125, 0:4],
                                scalar1=0.0, scalar2=None, op0=OP.max)

        def finish(src_ap):
            o_sb = work.tile([1, 25], dt)
            nc.vector.memset(o_sb[0:1, 0:25], 0.0)
            nc.vector.tensor_copy(o_sb[0:1, 0:src_ap.shape[1]], src_ap)
            nc.sync.dma_start(out=out_d.ap(), in_=o_sb[0:1, 0:25])

        if trunc == "l1":
            finish(x_sb[0:1, 0:4])
            nc.compile()
            return nc

        def gru_cell(idx, h_col, whh_sb, whh_off, wih_chunks, h_out):
            """One GRU cell.  h_col: smalls column holding [h; 1].
            whh_sb/whh_off: SBUF tile + col offset of [Whh^T; bhh^T] [88,261].
            wih_chunks: list of (lhsT_ap_fn, rhs_ap, K) for the input-side
            accumulation, lhsT_ap_fn(gate) -> AP.  h_out: [88,1] tile."""
            ps_rz = psum.tile([87, 2], dt, tag="ps_rz")
            ps_nin = psum.tile([87, 2], dt, tag="ps_nin")  # col0=i_n col1=h_n
            h_ap = smalls[0:87, h_col:h_col + 1]
            haug_ap = smalls[0:88, h_col:h_col + 1]

            # r group (wih chunks first, whh last — keeps early PE waits
            # off the whh DMA semaphore)
            for i, (lhsT_fn, rhs_ap, K) in enumerate(wih_chunks):
                nc.tensor.matmul(out=ps_rz[0:87, 0:1], lhsT=lhsT_fn(0),
                                 rhs=rhs_ap, start=(i == 0), stop=False)
            nc.tensor.matmul(out=ps_rz[0:87, 0:1],
                             lhsT=whh_sb[0:88, whh_off + 0:whh_off + 87],
                             rhs=haug_ap, start=False, stop=True)
            # i_n group (input side only; bias folded in wih)
            for i, (lhsT_fn, rhs_ap, K) in enumerate(wih_chunks):
                nc.tensor.matmul(out=ps_nin[0:87, 0:1], lhsT=lhsT_fn(2),
                                 rhs=rhs_ap, start=(i == 0),
                                 stop=(i == len(wih_chunks) - 1))
            # h_n group (hidden side only; bias bhh_n folded in whh)
            nc.tensor.matmul(out=ps_nin[0:87, 1:2],
                             lhsT=whh_sb[0:88, whh_off + 174:whh_off + 261],
                             rhs=haug_ap, start=True, stop=True)
            # z group
            for i, (lhsT_fn, rhs_ap, K) in enumerate(wih_chunks):
                nc.tensor.matmul(out=ps_rz[0:87, 1:2], lhsT=lhsT_fn(1),
                                 rhs=rhs_ap, start=(i == 0), stop=False)
            nc.tensor.matmul(out=ps_rz[0:87, 1:2],
                             lhsT=whh_sb[0:88, whh_off + 87:whh_off + 174],
                             rhs=haug_ap, start=False, stop=True)

            rz = work.tile([87, 2], dt, tag=f"rz{idx}")
            n_sb = work.tile([87, 1], dt, tag=f"n{idx}")
            d_sb = work.tile([87, 1], dt, tag=f"d{idx}")
            in_sb = work.tile([87, 1], dt, tag=f"in{idx}")
            # stage P_in into SBUF (ACT bias must be SBUF); overlaps sigmoid
            nc.vector.tensor_copy(in_sb[0:87, 0:1], ps_nin[0:87, 0:1])
            if use_act:
                # r = sigmoid(P_r)
                nc.scalar.activation(out=rz[0:87, 0:1], in_=ps_rz[0:87, 0:1],
                                     func=AF.Sigmoid)
                # n = tanh(r * P_hn + P_in)
                nc.scalar.activation(out=n_sb[0:87, 0:1],
                                     in_=ps_nin[0:87, 1:2],
                                     func=AF.Tanh, scale=rz[0:87, 0:1],
                                     bias=in_sb[0:87, 0:1])
                # z = sigmoid(P_z)
                nc.scalar.activation(out=rz[0:87, 1:2], in_=ps_rz[0:87, 1:2],
                                     func=AF.Sigmoid)
                # h' = n + z*(h - n), kept on ACT (no cross-engine hop):
                # negn = -n; d = h + negn; h' = z*d + n
                negn = work.tile([87, 1], dt, tag=f"negn{idx}")
                nc.scalar.activation(out=negn[0:87, 0:1], in_=n_sb[0:87, 0:1],
                                     func=AF.Identity, scale=-1.0)
                nc.scalar.activation(out=d_sb[0:87, 0:1], in_=h_ap,
                                     func=AF.Identity,
                                     bias=negn[0:87, 0:1])
                nc.scalar.activation(out=h_out[0:87, 0:1],
                                     in_=d_sb[0:87, 0:1], func=AF.Identity,
                                     scale=rz[0:87, 1:2],
                                     bias=n_sb[0:87, 0:1])
            else:
                nc.vector.tensor_copy(rz[0:87, 0:2], ps_rz[0:87, 0:2])
                nc.vector.tensor_copy(n_sb[0:87, 0:1], ps_nin[0:87, 1:2])
                nc.vector.tensor_sub(d_sb[0:87, 0:1], h_ap, n_sb[0:87, 0:1])
                nc.vector.scalar_tensor_tensor(out=h_out[0:87, 0:1],
                                               in0=d_sb[0:87, 0:1],
                                               scalar=rz[0:87, 1:2],
                                               in1=n_sb[0:87, 0:1],
                                               op0=OP.mult, op1=OP.add)

        # --- cell 0: x (501 = 125*3 + 126 chunks), h = smalls col 1 -----
        def wih0_ap(g, c, K):
            if g < 2:
                return wihrz[0:K, 348 * g + 87 * c:348 * g + 87 * c + 87]
            return wihn[0:K, 87 * c:87 * c + 87]

        wih0_chunks = []
        for c in range(4):
            K = 125 if c < 3 else 126
            rhs = x_sb[0:K, c:c + 1]
            wih0_chunks.append(
                (lambda g, c=c, K=K: wih0_ap(g, c, K), rhs, K))
        gru_cell(0, 1, whh0, 0, wih0_chunks, h10)

        if trunc == "cell0":
            finish(h10[0:1, 0:1])
            nc.compile()
            return nc

        # --- cell 1: h10 (88 incl one), h = smalls col 2 ----------------
        wih1_chunks = [
            (lambda g: tail[0:88, 261 + 87 * g:261 + 87 * g + 87],
             h10[0:88, 0:1], 88)]
        gru_cell(1, 2, tail, 0, wih1_chunks, h11)

        if trunc == "cell1":
            finish(h11[0:1, 0:1])
            nc.compile()
            return nc

        # --- L2a: p = relu(W2a @ h11 + b2a) -----------------------------
        ps_p = psum.tile([40, 1], dt, tag="l2")
        nc.tensor.matmul(out=ps_p[0:40, 0:1], lhsT=tail[0:88, 522:562],
                         rhs=h11[0:88, 0:1], start=True, stop=True)
        nc.vector.tensor_scalar(out=p_sb[0:40, 0:1], in0=ps_p[0:40, 0:1],
                                scalar1=0.0, scalar2=None, op0=OP.max)

        # --- L2b: out = (p^T @ W2b^T) as a [1,25] row so the output DMA
        # is a single contiguous descriptor ------------------------------
        ps_o = psum.tile([1, 25], dt, tag="l2")
        nc.tensor.matmul(out=ps_o[0:1, 0:25], lhsT=p_sb[0:41, 0:1],
                         rhs=tail[0:41, 562:587], start=True, stop=True)
        o_sb = work.tile([1, 25], dt)
        nc.vector.tensor_copy(o_sb[0:1, 0:25], ps_o[0:1, 0:25])
        nc.sync.dma_start(out=out_d.ap(), in_=o_sb[0:1, 0:25])

    nc.compile()
    return nc


# ---------------------------------------------------------------------------
# Host-side input packing
# ---------------------------------------------------------------------------

def _aug(wT, b):
    """[W^T; b^T] bias augmentation."""
    return np.vstack([np.asarray(wT, F32), np.asarray(b, F32)[None, :]])


def pack_branch(in_parts, W1, b1, wih0, whh0, bih0, bhh0, wih1, whh1, bih1,
                bhh1, W2a, b2a, W2b, b2b, hn):
    """Build the per-core input map for one branch (np arrays, branch-1
    shapes; caller zero-pads branch 2)."""
    in_vec = np.concatenate([np.concatenate(in_parts), [1.0]]).astype(F32)
    smalls = np.zeros((128, 3), F32)
    smalls[:23, 0] = in_vec
    smalls[:87, 1] = hn[0]
    smalls[87, 1] = 1.0
    smalls[:87, 2] = hn[1]
    smalls[87, 2] = 1.0

    w1t = _aug(W1.T, b1)                               # [23, 500]
    whh0_a = _aug(whh0.T, bhh0)                        # [88, 261]
    wih0_a = _aug(wih0.T, bih0)                        # [501, 261]
    # gate-major chunk pack: col block g*348 + c*87 holds K-chunk c of gate g
    wih0_p = np.zeros((126, 1044), F32)
    for g in range(3):
        for c in range(4):
            r0, kn = 125 * c, (125 if c < 3 else 126)
            wih0_p[0:kn, 348 * g + 87 * c:348 * g + 87 * c + 87] = \
                wih0_a[r0:r0 + kn, 87 * g:87 * g + 87]

    tail = np.zeros((88, 587), F32)
    tail[:, 0:261] = _aug(whh1.T, bhh1)
    tail[:, 261:522] = _aug(wih1.T, bih1)
    tail[:, 522:562] = _aug(W2a.T, b2a)
    tail[0:41, 562:587] = _aug(W2b.T, b2b)

    return {"smalls": smalls, "w1t": w1t, "whh0": whh0_a,
            "wihrz": np.ascontiguousarray(wih0_p[:, 0:696]),
            "wihn": np.ascontiguousarray(wih0_p[:, 696:1044]),
            "tail": tail}


def make_in_maps(inputs):
    i = {k: np.asarray(v, F32) for k, v in inputs.items()}

    m0 = pack_branch(
        [i["state_inno"], i["diff_state"], i["linearization_error"],
         i["Jacobian"]],
        i["W_l1"], i["b_l1"],
        i["gru1_wih0"], i["gru1_whh0"], i["gru1_bih0"], i["gru1_bhh0"],
        i["gru1_wih1"], i["gru1_whh1"], i["gru1_bih1"], i["gru1_bhh1"],
        i["W_l2a"], i["b_l2a"], i["W_l2b"], i["b_l2b"], i["hn1"])

    # branch 2: pad 16-dim input to 22 and [4,40] head to [25,40]
    W3p = np.vstack([i["W_l3"].T, np.zeros((6, 500), F32)]).T  # [500, 22]
    W4bp = np.zeros((25, 40), F32)
    W4bp[:4] = i["W_l4b"]
    b4bp = np.zeros(25, F32)
    b4bp[:4] = i["b_l4b"]
    m1 = pack_branch(
        [i["observation_inno"], i["diff_obs"], i["linearization_error"],
         i["Jacobian"], np.zeros(6, F32)],
        W3p, i["b_l3"],
        i["gru2_wih0"], i["gru2_whh0"], i["gru2_bih0"], i["gru2_bhh0"],
        i["gru2_wih1"], i["gru2_whh1"], i["gru2_bih1"], i["gru2_bhh1"],
        i["W_l4a"], i["b_l4a"], W4bp, b4bp, i["hn2"])

    return [m0, m1] + [m0] * (N_CORES - 2)


# ---------------------------------------------------------------------------
# Entry point
# ---------------------------------------------------------------------------

def kernel(**inputs):
    from concourse.bass_utils import run_bass_kernel_spmd

    if "nc" not in _RT:
        _RT["nc"] = build_nc()
    in_maps = make_in_maps(inputs)
    res = run_bass_kernel_spmd(_RT["nc"], in_maps, list(range(N_CORES)))
    Pk = res.results[0]["out"][0, :].reshape(X_DIM, X_DIM).astype(F32)
    Sk = res.results[1]["out"][0, 0:4].reshape(Y_DIM, Y_DIM).astype(F32)
    return (Pk, Sk)
